# revision 49
# baseline (speedup 1.0000x reference)
"""CrossAttnBlock kernel for 8 Trainium2 NeuronCores.

Sharding: core c -> (batch b = c//2, token-half s = c%2), 512 query tokens
per core. K/V for both attentions are computed only for the core's OWN 512
tokens and exchanged with PAIR-wise AllGathers (bf16): each core stores its
own half at k-token slots 0:512 and the partner half at 512:1024 (softmax
is permutation-invariant over keys, so no parity logic is needed).

All activations are feature-major ([feature, token]). Weights are pre-tiled
on the host into contiguous [nt, 128, C8*128] bf16 blocks so each weight
DMA is a single large contiguous transfer. LayerNorm affine params are
folded into the weights on the host (diag(s) @ W row-scaling); the LN bias
contributions become per-output-feature biases (K-projection bias drops
exactly -- softmax is invariant to per-query score shifts; V-projection bias
passes through the normalized softmax unchanged and folds into the output
projection bias). K/V/x2 stay SBUF-resident.

Attention softmax: scores for 2 k-tile blocks land in one [128,1024] PSUM
tile (2 banks) and are exponentiated by a single Scalar ACTIVATE (the
~352-cycle pipeline fill amortizes over 1024 columns). Heads are
software-pipelined: head h's score matmuls and EXPs are emitted before head
h-1's 8 back-to-back o_ps accumulation matmuls, so the PE never waits on
the Scalar engine and the o_ps accumulation hits consecutive-same-bank
speed. The softmax 1/z row uses a ones-column in V, reciprocal directly
from PSUM on DVE, a GpSimd partition_broadcast, and one DVE multiply.
"""
import sys

sys.path.insert(0, '/opt/trn_rl_repo')

import ml_dtypes
import numpy as np
import concourse.bass as bass
from concourse import bacc
import concourse.tile as tile
from concourse import mybir
from concourse import bass_isa

F32R = mybir.dt.float32r
F32 = mybir.dt.float32
BF16 = mybir.dt.bfloat16
AF = mybir.ActivationFunctionType
OP = mybir.AluOpType

N_CORES = 8
B, NSEQ, D, H, HD = 4, 1024, 1024, 16, 64
T = 512            # tokens owned per core
TF = 1024          # full token count per batch
C8 = D // 128      # feature chunks
SCALE = 1.0 / float(np.sqrt(np.float32(HD)))
EPS = 1e-6

_PROGRAM_CACHE = {}
import os
KDBG = os.environ.get("KDBG", "")


def _build_program():
    nc = bacc.Bacc("TRN2", target_bir_lowering=False, debug=False,
                   num_devices=N_CORES)

    dp = {}
    dp["xT"] = nc.declare_dram_parameter("xT", [128, C8, T], F32R,
                                         isOutput=False)
    dp["xbfT"] = nc.declare_dram_parameter("xbfT", [128, C8, T], BF16,
                                           isOutput=False)
    dp["kvT"] = nc.declare_dram_parameter("kvT", [128, C8, T], BF16,
                                          isOutput=False)
    # pre-tiled bf16 weights: [nt, 128, C8(contraction chunks), 128]
    for nm, ntiles, nchunk in [("wq", 8, 8), ("wk1", 8, 8), ("wco", 8, 8),
                               ("wq2", 8, 8), ("wk2", 8, 8), ("wso", 8, 8),
                               ("w1", 32, 8), ("w2", 8, 32)]:
        dp[nm] = nc.declare_dram_parameter(nm, [ntiles, 128, nchunk, 128],
                                           BF16, isOutput=False)
    # V-projection weights in moving layout [128, C8, 1024]
    dp["wv1"] = nc.declare_dram_parameter("wv1", [128, C8, TF], BF16,
                                          isOutput=False)
    dp["wv2"] = nc.declare_dram_parameter("wv2", [128, C8, TF], BF16,
                                          isOutput=False)
    # all per-partition column params packed into one [128, 146] f32 tensor
    # bqp1[16] bqp2[16] mask2[2] bco[8] bso[8] b1[32] b2[8] csk1[8] csk2[8]
    # csw1[32] csq1[8]
    dp["cols"] = nc.declare_dram_parameter("cols", [128, 146], F32,
                                           isOutput=False)
    dp["outT"] = nc.declare_dram_parameter("outT", [128, C8, T], F32R,
                                           isOutput=True)

    with tile.TileContext(nc) as tc:
        _emit(nc, tc, dp)
    nc.compile()
    return nc


def _emit(nc, tc, dp):
    import contextlib

    ctx = contextlib.ExitStack()
    with ctx:
        consts = ctx.enter_context(tc.tile_pool(name="consts", bufs=1))
        outer = ctx.enter_context(tc.tile_pool(name="outer", bufs=1))
        work = ctx.enter_context(tc.tile_pool(name="work", bufs=1))
        pp = ctx.enter_context(tc.tile_pool(name="pp", bufs=1, space="PSUM"))
        small = ctx.enter_context(tc.tile_pool(name="small", bufs=1))
        dramp = ctx.enter_context(tc.tile_pool(name="dramp", bufs=1,
                                               space="DRAM"))

        # ---------- constants ----------
        ones_bf = consts.tile([128, 128], BF16)
        nc.vector.memset(ones_bf[:], 1.0)
        eps_t = consts.tile([1, 1], F32)
        nc.vector.memset(eps_t[:], EPS)
        eps_p = consts.tile([128, 1], F32)
        nc.vector.memset(eps_p[:], EPS)

        cols_sb = consts.tile([128, 146], F32, name="cols_sb")
        nc.sync.dma_start(out=cols_sb[:], in_=dp["cols"][:])
        _off = [0]

        def take_col(n):
            c = cols_sb[:, _off[0]:_off[0] + n]
            _off[0] += n
            return c

        bqp1_c = take_col(H)
        bqp2_c = take_col(H)
        mask2_c = take_col(2)
        bco_c = take_col(C8)
        bso_c = take_col(C8)
        b1_c = take_col(32)
        b2_c = take_col(C8)
        csk1_c = take_col(C8)
        csk2_c = take_col(C8)
        csw1_c = take_col(32)
        csq1_c = take_col(C8)

        pid = nc.sync.partition_id()
        partner_slot = 1 - pid % 2

        # ---------- DRAM intermediates (K/V pair-exchange buffers) ------
        # Two collectives per exchange: >=2MB payloads switch the collective
        # from the fast MESH algorithm to a slow RING -- keep each ~1MB.
        KSZ = C8 * T            # 4096 elems: own-half K, feature-major
        VSZ = 4 * H * 65        # 4160 elems: own-half V (incl ones col)
        kpack = [dramp.tile([128, KSZ], BF16, name=f"kpack{i}")
                 for i in range(2)]
        agk = [dramp.tile([2, 128, KSZ], BF16, name=f"agk{i}")
               for i in range(2)]
        vpack = [dramp.tile([128, VSZ], BF16, name=f"vpack{i}")
                 for i in range(2)]
        agv = [dramp.tile([2, 128, VSZ], BF16, name=f"agv{i}")
               for i in range(2)]
        PAIRS = [[0, 1], [2, 3], [4, 5], [6, 7]]

        # tiny rendezvous at t=0: absorbs the pair launch skew while the
        # PE is still idle, so the real K/V exchanges don't pay it.
        # (collectives cannot read IO tensors -- stage via a DRAM tile)
        presync_src = dramp.tile([1, 8], F32, name="presync_src")
        nc.sync.dma_start(out=presync_src[:], in_=cols_sb[0:1, 0:8])
        presync_ag = dramp.tile([2, 1, 8], F32, name="presync_ag")
        nc.gpsimd.collective_compute(
            "AllGather", OP.bypass, ins=[presync_src[:]],
            outs=[presync_ag[:]], replica_groups=PAIRS)

        def send_k(i, kT_sb):
            """Pack (Activation HWDGE queue; its deps are already satisfied
            at dispatch so it never head-of-line blocks Scalar compute) and
            launch the K AllGather."""
            nc.scalar.dma_start(out=kpack[i][:], in_=kT_sb[:, 0, :, :])
            nc.gpsimd.collective_compute(
                "AllGather", OP.bypass, ins=[kpack[i][:]],
                outs=[agk[i][:]], replica_groups=PAIRS)

        def send_v(i, v_sb):
            nc.scalar.dma_start(out=vpack[i][:], in_=v_sb[:, 0:4, :, :])
            nc.gpsimd.collective_compute(
                "AllGather", OP.bypass, ins=[vpack[i][:]],
                outs=[agv[i][:]], replica_groups=PAIRS)

        def recv_kv(i, kT_sb, v_sb):
            """Land the partner half at k-token slots 512:1024. Emitted as
            the LAST Q1 entries before the attention that consumes them so
            their wait on the collective doesn't block independent loads."""
            nc.sync.dma_start(
                out=kT_sb[:, 1, :, :],
                in_=agk[i][bass.ds(partner_slot, 1), :, :].rearrange(
                    "o p (c t) -> p (o c) t", c=C8))
            nc.sync.dma_start(
                out=v_sb[:, 4:8, :, :],
                in_=agv[i][bass.ds(partner_slot, 1), :, :].rearrange(
                    "o p (j h e) -> p (o j) h e", j=4, h=H))

        # ---------- generic helpers ----------
        def layer_norm(src_fn, dst, dst_sl, pool):
            """Pure LN (no affine) over the feature axis for 512 tokens.

            src_fn(c) -> [128, 512] bf16 AP. dst: [128, C8, *] SBUF tile.
            """
            stats = pp.tile([128, 1024], F32, tag="big", bufs=2,
                            name="stats")
            srcs = []
            for c in range(C8):
                xc = src_fn(c)
                srcs.append(xc)
                sq = work.tile([128, 512], BF16, tag="sq", bufs=3, name="sq")
                nc.scalar.activation(out=sq[:], in_=xc, func=AF.Square)
                nc.tensor.matmul(stats[:, 0:512], ones_bf[:, :], xc,
                                 start=(c == 0), stop=(c == C8 - 1),
                                 skip_group_check=True)
                nc.tensor.matmul(stats[:, 512:1024], ones_bf[:, :], sq[:],
                                 start=(c == 0), stop=(c == C8 - 1),
                                 skip_group_check=True)
            # every row of stats is the full reduction (ones stationary), so
            # the mean/var/rsqrt tail runs at [128,512] and no PE broadcast
            # or cast is needed before the 2x-mode bf16 normalize.
            mean = work.tile([128, 512], F32, tag="lnmean", bufs=1,
                             name="mean")
            nc.vector.tensor_scalar_mul(mean[:], stats[:, 0:512], 1.0 / D)
            m2 = work.tile([128, 512], F32, tag="lnm2", bufs=1, name="m2")
            nc.vector.tensor_mul(m2[:], mean[:], mean[:])
            var = work.tile([128, 512], F32, tag="lnvar", bufs=1, name="var")
            nc.vector.scalar_tensor_tensor(
                out=var[:], in0=stats[:, 512:1024], scalar=1.0 / D,
                in1=m2[:], op0=OP.mult, op1=OP.subtract)
            std = work.tile([128, 512], F32, tag="lnstd", bufs=1,
                            name="std")
            nc.scalar.activation(out=std[:], in_=var[:], func=AF.Sqrt,
                                 bias=eps_p[:])
            inv = work.tile([128, 512], F32, tag="lninv", bufs=1, name="inv")
            nc.vector.reciprocal_approx_fast(out=inv[:], in_=std[:])
            a0b = work.tile([128, 512], BF16, tag="a0b", bufs=2, name="a0b")
            nc.vector.tensor_copy(out=a0b[:], in_=inv[:])
            c0b = work.tile([128, 512], BF16, tag="c0b", bufs=2, name="c0b")
            nc.vector.scalar_tensor_tensor(
                out=c0b[:], in0=mean[:], scalar=-1.0, in1=a0b[:],
                op0=OP.mult, op1=OP.mult)
            if dst is not None:
                for c in range(C8):
                    nc.vector.tensor_mul(dst[:, c, dst_sl], srcs[c], a0b[:])
                    nc.vector.tensor_add(dst[:, c, dst_sl],
                                         dst[:, c, dst_sl], c0b[:])
            return a0b, c0b

        def gemm_feat(w_dram, n_tiles, rhs_list, evict):
            """Feature-major GEMM. w_dram: [nt, 128, C8, 128] bf16 tiles.
            rhs_list: [(rhs_fn(c) -> [128,512] AP, key)]. evict(nt, key, ps)."""
            for nt in range(n_tiles):
                wt = work.tile([128, C8, 128], BF16, tag="wt", bufs=3,
                               name="wt")
                nc.sync.dma_start(out=wt[:], in_=w_dram[nt])
                for (rhs_fn, key) in rhs_list:
                    ps = pp.tile([128, 512], F32, tag="mm", bufs=2, name="gps")
                    for c in range(C8):
                        nc.tensor.matmul(ps[:], wt[:, c, :], rhs_fn(c),
                                         start=(c == 0), stop=(c == C8 - 1))
                    evict(nt, key, ps)

        def build_v(src, jts, wv_sb, v_sb):
            """Token-major V into SBUF. src: [128, C8, 512] bf16 (LN out);
            jts: j-tile indices (token blocks of 128) relative to src.
            v_sb: [128, 8, H, 65] (col 64 of each head slot holds ones)."""
            for jt in jts:
                sl = slice((jt % 4) * 128, (jt % 4 + 1) * 128)
                vps0 = pp.tile([128, 512], F32, tag="ops", bufs=2,
                               name="vps0")
                vps1 = pp.tile([128, 512], F32, tag="ops", bufs=2,
                               name="vps1")
                for c in range(C8):
                    lhsT = src[:, c, sl]
                    nc.tensor.matmul(vps0[:], lhsT, wv_sb[:, c, 0:512],
                                     start=(c == 0), stop=(c == C8 - 1),
                                     skip_group_check=True)
                    nc.tensor.matmul(vps1[:], lhsT,
                                     wv_sb[:, c, 512:1024],
                                     start=(c == 0), stop=(c == C8 - 1),
                                     skip_group_check=True)
                nc.vector.tensor_copy(
                    out=v_sb[:, jt, 0:8, 0:64],
                    in_=vps0[:].rearrange("p (h e) -> p h e", h=8))
                nc.vector.tensor_copy(
                    out=v_sb[:, jt, 8:16, 0:64],
                    in_=vps1[:].rearrange("p (h e) -> p h e", h=8))

        def attention(qT, kT_sb, v_sb, oT, opart, aname=""):
            """kT_sb: [128, 2, C8, 512] (half-major); v_sb: [128, 8, H, 65].
            jt 0-3 = token half 0 (own), jt 4-7 = half 1 (partner). Heads
            are software-pipelined (head h's scores+EXPs emitted before the
            previous flush's o_ps accumulation, so the PE never waits on the
            Scalar EXPs). The first SH heads use SPLIT accumulation: pass A
            computes own-half partial o/z into `opart` with NO partner
            dependency, absorbing the K/V AllGather latency; pass B adds
            the partner half and evicts."""
            SH = 8

            def score_batch(h, bb):
                ch = h // 2
                s_big = pp.tile([128, 1024], F32, tag="big", bufs=2,
                                name="s_big")
                for k in range(2):
                    jt = bb * 2 + k
                    nc.tensor.matmul(
                        s_big[:, k * 512:(k + 1) * 512],
                        kT_sb[:, jt // 4, ch,
                              (jt % 4) * 128:(jt % 4 + 1) * 128],
                        qT[:, h, :], start=True, stop=True)
                pt = work.tile([128, 1024], BF16, tag="pt", bufs=6,
                               name="pt")
                nc.scalar.activation(out=pt[:], in_=s_big[:],
                                     func=AF.Exp, scale=SCALE)
                return pt

            def accum(o_ps, pts, h, jts):
                for i, jt in enumerate(jts):
                    nc.tensor.matmul(o_ps[0:65, :], v_sb[:, jt, h, :],
                                     pts[i // 2][:, (i % 2) * 512:
                                                 (i % 2 + 1) * 512],
                                     start=(i == 0), stop=(i == len(jts) - 1),
                                     skip_group_check=True)

            def evict(o_ps, h, partial):
                """partial: opart slice to fold in (pass B), else None."""
                ch, off = h // 2, (h % 2) * 64
                if partial is not None:
                    tsum = work.tile([128, 512], F32, tag="tsum", bufs=2,
                                     name="tsum")
                    nc.vector.tensor_add(tsum[0:65, :], o_ps[0:65, :],
                                         partial[0:65, :])
                    src = tsum
                else:
                    src = o_ps
                zrow = small.tile([1, 512], F32, tag="zrow", bufs=2,
                                  name="zrow")
                nc.vector.tensor_copy(out=zrow[:], in_=src[64:65, :])
                zrec = small.tile([1, 512], F32, tag="zrec", bufs=2,
                                  name="zrec")
                nc.vector.reciprocal_approx_fast(out=zrec[:], in_=zrow[:])
                zbc = work.tile([64, 512], F32, tag="zbc", bufs=2,
                                name="zbc")
                nc.gpsimd.partition_broadcast(zbc[:], zrec[:], channels=64)
                nc.vector.tensor_mul(oT[off:off + 64, ch, :],
                                     src[0:64, :], zbc[:])

            def do_flush(p):
                kind, o_ps, pts, h = p
                if kind == 'A':
                    accum(o_ps, pts, h, [0, 1, 2, 3])
                    nc.vector.tensor_copy(out=opart[0:65, h, :],
                                          in_=o_ps[0:65, :])
                elif kind == 'B':
                    accum(o_ps, pts, h, [4, 5, 6, 7])
                    evict(o_ps, h, opart[:, h, :])
                else:
                    accum(o_ps, pts, h, list(range(8)))
                    evict(o_ps, h, None)

            pend = None
            for h in range(SH):
                pts = [score_batch(h, 0), score_batch(h, 1)]
                o_ps = pp.tile([128, 512], F32, tag="ops", bufs=2,
                               name="o_ps")
                if pend is not None:
                    do_flush(pend)
                pend = ('A', o_ps, pts, h)
            for h in range(SH):
                pts = [score_batch(h, 2), score_batch(h, 3)]
                o_ps = pp.tile([128, 512], F32, tag="ops", bufs=2,
                               name="o_ps")
                do_flush(pend)
                pend = ('B', o_ps, pts, h)
            for h in range(SH, H):
                pts = [score_batch(h, bb) for bb in range(4)]
                o_ps = pp.tile([128, 512], F32, tag="ops", bufs=2,
                               name="o_ps")
                do_flush(pend)
                pend = ('F', o_ps, pts, h)
            do_flush(pend)

        def dbg_dump(name, ap_fn, n=C8, bf=False):
            """If KDBG==name, copy chunks into outT."""
            if KDBG != name:
                return False
            for c in range(n):
                st = work.tile([128, 512], F32R, tag="dbgst", bufs=2,
                               name="dbgst")
                nc.scalar.copy(out=st[:], in_=ap_fn(c))
                nc.sync.dma_start(out=dp["outT"][:, c % C8, :], in_=st[:])
            return True

        # x1/xbf DMAs are emitted inside phase 1 (after the K/V exchange is
        # queued) so the latency-critical kv-side loads go first on Q1.
        x1 = outer.tile([128, C8, T], F32R, tag="res", bufs=2, name="x1")
        xbf = outer.tile([128, C8, T], BF16, tag="resbf", bufs=2, name="xbf")

        def ev_qT(qTp, col):
            def ev(nt, key, ps):
                for k in (0, 1):
                    h = 2 * nt + k
                    nc.vector.tensor_scalar(
                        out=qTp[:, h, :], in0=ps[:],
                        scalar1=mask2_c[:, k:k + 1],
                        scalar2=col[:, h:h + 1],
                        op0=OP.mult, op1=OP.add)
            return ev

        def ev_qT_aff(qTp, col, a0b, c0b, cs):
            """Q ran on the RAW (pre-LN) input: apply the LN affine on DVE,
            then the head-half mask + folded bias on Scalar."""
            def ev(nt, key, ps):
                t = work.tile([128, 512], BF16, tag="qaff", bufs=2,
                              name="qaff")
                nc.vector.tensor_mul(t[:], ps[:], a0b[:])
                t2 = work.tile([128, 512], BF16, tag="qaff2", bufs=2,
                               name="qaff2")
                nc.vector.scalar_tensor_tensor(
                    out=t2[:], in0=c0b[:], scalar=cs[:, nt:nt + 1],
                    in1=t[:], op0=OP.mult, op1=OP.add)
                for k in (0, 1):
                    h = 2 * nt + k
                    nc.scalar.activation(
                        out=qTp[:, h, :], in_=t2[:], func=AF.Identity,
                        scale=mask2_c[:, k:k + 1], bias=col[:, h:h + 1])
            return ev

        def ev_kT_aff(kTp, a0b, c0b, cs):
            """K-projection ran on the RAW (pre-LN) input; apply the
            per-token LN affine post-GEMM: K = P*inv + (-m*inv)*colsum(W)."""
            def ev(nt, th, ps):
                t = work.tile([128, 512], F32, tag="kaff", bufs=2, name="kaff")
                nc.vector.tensor_mul(t[:], ps[:], a0b[:])
                nc.vector.scalar_tensor_tensor(
                    out=kTp[:, th, nt, :], in0=c0b[:],
                    scalar=cs[:, nt:nt + 1], in1=t[:],
                    op0=OP.mult, op1=OP.add)
            return ev

        # ================= Phase 1: cross-attention =================
        with tc.tile_pool(name="p1", bufs=1) as p1:
            kvh0 = p1.tile([128, C8, 512], BF16, tag="kvh", bufs=1,
                           name="kvh")
            for qq in range(4):
                nc.sync.dma_start(out=kvh0[:, 2 * qq:2 * qq + 2, :],
                                  in_=dp["kvT"][:, 2 * qq:2 * qq + 2, :])
            kv_in = p1.tile([128, C8, T], BF16, name="kv_in")
            ab_kv, cb_kv = layer_norm(lambda c: kvh0[:, c, :], kv_in,
                                      slice(0, 512), p1)

            kT_sb = p1.tile([128, 2, C8, 512], BF16, name="kT_sb")
            v_sb = p1.tile([128, 8, H, 65], BF16, name="v_sb")
            nc.vector.memset(v_sb[:, :, :, 64], 1.0)
            wv_sb = work.tile([128, C8, TF], BF16, tag="wv", bufs=1,
                              name="wv_sb")
            nc.sync.dma_start(out=wv_sb[:], in_=dp["wv1"][:])
            wv2_sb = work.tile([128, C8, TF], BF16, tag="wv", bufs=1,
                               name="wv2_sb")
            gemm_feat(dp["wk1"], C8,
                      [(lambda c: kvh0[:, c, :], 0)],
                      ev_kT_aff(kT_sb, ab_kv, cb_kv, csk1_c))
            send_k(0, kT_sb)
            build_v(kv_in, range(0, 4), wv_sb, v_sb)
            send_v(0, v_sb)

            nc.sync.dma_start(out=xbf[:], in_=dp["xbfT"][:])
            nc.sync.dma_start(out=x1[:], in_=dp["xT"][:])
            ab_q, cb_q = layer_norm(lambda c: xbf[:, c, :], None, None, p1)

            qT1 = p1.tile([128, H, T], BF16, name="qT1")
            gemm_feat(dp["wq"], C8, [(lambda c: xbf[:, c, :], 0)],
                      ev_qT_aff(qT1, bqp1_c, ab_q, cb_q, csq1_c))

            nc.sync.dma_start(out=wv2_sb[:], in_=dp["wv2"][:])
            recv_kv(0, kT_sb, v_sb)
            oT1 = p1.tile([128, C8, T], BF16, name="oT1")
            opart1 = p1.tile([128, 8, 512], BF16, tag="kvh", bufs=1,
                             name="opart1")
            attention(qT1, kT_sb, v_sb, oT1, opart1, aname="a1")

            # x2 = x1 + Wco @ o + bco'
            x2 = outer.tile([128, C8, T], F32R, tag="res", bufs=2, name="x2")
            x2bf = outer.tile([128, C8, T], BF16, tag="resbf", bufs=2,
                              name="x2bf")

            def ev_x2(nt, key, ps):
                nc.vector.scalar_tensor_tensor(
                    out=x2[:, nt, :], in0=ps[:], scalar=bco_c[:, nt:nt + 1],
                    in1=x1[:, nt, :], op0=OP.add, op1=OP.add)
                nc.scalar.activation(out=x2bf[:, nt, :], in_=x2[:, nt, :],
                                     func=AF.Copy)

            gemm_feat(dp["wco"], C8, [(lambda c: oT1[:, c, :], 0)], ev_x2)
            dbg_dump("kv_in", lambda c: kv_in[:, c, :])
            dbg_dump("oT1", lambda c: oT1[:, c, :])
            dbg_dump("x2", lambda c: x2[:, c, :])

        # ================= Phase 2: self-attention =================
        with tc.tile_pool(name="p2", bufs=1) as p2:
            s_own = p2.tile([128, C8, T], BF16, name="s_own")
            ab_s, cb_s = layer_norm(lambda c: x2bf[:, c, :], s_own,
                                    slice(0, 512), p2)

            kT2_sb = p2.tile([128, 2, C8, 512], BF16, name="kT2_sb")
            v2_sb = p2.tile([128, 8, H, 65], BF16, name="v2_sb")
            nc.vector.memset(v2_sb[:, :, :, 64], 1.0)
            gemm_feat(dp["wk2"], C8, [(lambda c: x2bf[:, c, :], 0)],
                      ev_kT_aff(kT2_sb, ab_s, cb_s, csk2_c))
            send_k(1, kT2_sb)
            build_v(s_own, range(0, 4), wv2_sb, v2_sb)
            send_v(1, v2_sb)

            qT2 = p2.tile([128, H, T], BF16, name="qT2")
            gemm_feat(dp["wq2"], C8, [(lambda c: s_own[:, c, :], 0)],
                      ev_qT(qT2, bqp2_c))

            recv_kv(1, kT2_sb, v2_sb)
            oT2 = p2.tile([128, C8, T], BF16, name="oT2")
            opart2 = p2.tile([128, 8, 512], BF16, name="opart2")
            attention(qT2, kT2_sb, v2_sb, oT2, opart2, aname="a2")

            x3 = outer.tile([128, C8, T], F32R, tag="res", bufs=2, name="x3")
            x3bf = outer.tile([128, C8, T], BF16, tag="resbf", bufs=2,
                              name="x3bf")

            def ev_x3(nt, key, ps):
                nc.vector.scalar_tensor_tensor(
                    out=x3[:, nt, :], in0=ps[:], scalar=bso_c[:, nt:nt + 1],
                    in1=x2[:, nt, :], op0=OP.add, op1=OP.add)
                nc.scalar.activation(out=x3bf[:, nt, :], in_=x3[:, nt, :],
                                     func=AF.Copy)

            gemm_feat(dp["wso"], C8, [(lambda c: oT2[:, c, :], 0)], ev_x3)
            dbg_dump("s_own", lambda c: s_own[:, c, :])
            dbg_dump("oT2", lambda c: oT2[:, c, :])
            dbg_dump("x3", lambda c: x3[:, c, :])

        # ================= Phase 3: MLP =================
        with tc.tile_pool(name="p3", bufs=1) as p3:
            ab_m, cb_m = layer_norm(lambda c: x3bf[:, c, :], None, None, p3)

            hT = p3.tile([128, 32, T], BF16, name="hT")

            def ev_h(ht, key, ps):
                t = work.tile([128, 512], F32, tag="kaff", bufs=2,
                              name="haff")
                nc.vector.tensor_mul(t[:], ps[:], ab_m[:])
                t2 = work.tile([128, 512], F32, tag="kaff2", bufs=2,
                               name="haff2")
                nc.vector.scalar_tensor_tensor(
                    out=t2[:], in0=cb_m[:], scalar=csw1_c[:, ht:ht + 1],
                    in1=t[:], op0=OP.mult, op1=OP.add)
                nc.scalar.activation(out=hT[:, ht, :], in_=t2[:],
                                     func=AF.Gelu_apprx_tanh,
                                     bias=b1_c[:, ht:ht + 1], scale=1.0)

            gemm_feat(dp["w1"], 32, [(lambda c: x3bf[:, c, :], 0)], ev_h)

            for nt in range(C8):
                w2t = p3.tile([128, 32, 128], BF16, tag="w2t", bufs=2,
                              name="w2t")
                nc.sync.dma_start(out=w2t[:], in_=dp["w2"][nt])
                ps = pp.tile([128, 512], F32, tag="mm", bufs=2, name="ops2")
                for kk in range(32):
                    nc.tensor.matmul(ps[:], w2t[:, kk, :], hT[:, kk, :],
                                     start=(kk == 0), stop=(kk == 31))
                ot = p3.tile([128, 512], F32R, tag="ot", bufs=2, name="ot")
                nc.vector.tensor_scalar_add(ot[:], ps[:],
                                            scalar1=b2_c[:, nt:nt + 1])
                if not KDBG:
                    nc.sync.dma_start(out=dp["outT"][:, nt, :], in_=ot[:])


def _get_program():
    if "nc" not in _PROGRAM_CACHE:
        _PROGRAM_CACHE["nc"] = _build_program()
    return _PROGRAM_CACHE["nc"]


def _tile_w(w):
    """[Din, NT*128] f32 -> [NT, 128, Din//128, 128] bf16 contiguous."""
    din, dout = w.shape
    cn, nt = din // 128, dout // 128
    t = w.reshape(cn, 128, nt, 128).transpose(2, 1, 0, 3)
    return np.ascontiguousarray(t).astype(ml_dtypes.bfloat16)


def _mov_w(w):
    """[Din, N] f32 -> [128, Din//128, N] bf16 (moving-operand layout)."""
    din, n = w.shape
    cn = din // 128
    t = w.reshape(cn, 128, n).transpose(1, 0, 2)
    return np.ascontiguousarray(t).astype(ml_dtypes.bfloat16)


def _headpad_bias(b):
    """[D] bias -> [128, H]: col h holds bias on its active 64 rows."""
    out = np.zeros((128, H), np.float32)
    for h in range(H):
        ch, off = h // 2, (h % 2) * 64
        out[off:off + 64, h] = b[ch * 128 + off:ch * 128 + off + 64]
    return out


def _halves_mask():
    m = np.zeros((128, 2), np.float32)
    m[0:64, 0] = 1.0
    m[64:128, 1] = 1.0
    return m


def _colmaj(v):
    """[n*128] bias -> [128, n]: column c holds features c*128..c*128+127."""
    return np.ascontiguousarray(v.reshape(-1, 128).T)


def _chunk_fm(a):
    """[n_tok, D] f32 -> feature-major chunked [128, C8, n_tok]."""
    ntok = a.shape[0]
    t = a.T.reshape(C8, 128, ntok).transpose(1, 0, 2)
    return np.ascontiguousarray(t)


def kernel(**inputs) -> np.ndarray:
    from concourse.bass_utils import run_bass_kernel_spmd

    nc = _get_program()

    f32 = lambda a: np.asarray(a, np.float32)
    x = f32(inputs["x"])
    key_val = f32(inputs["key_val"])
    ln1_s, ln1_b = f32(inputs["ln1_s"]), f32(inputs["ln1_b"])
    ln2_s, ln2_b = f32(inputs["ln2_s"]), f32(inputs["ln2_b"])
    ln3_s, ln3_b = f32(inputs["ln3_s"]), f32(inputs["ln3_b"])
    ln4_s, ln4_b = f32(inputs["ln4_s"]), f32(inputs["ln4_b"])
    Wq, Wkv, Wco = f32(inputs["Wq"]), f32(inputs["Wkv"]), f32(inputs["Wco"])
    Wqkv, Wso = f32(inputs["Wqkv"]), f32(inputs["Wso"])
    W1, W2 = f32(inputs["W1"]), f32(inputs["W2"])
    bco, bso = f32(inputs["bco"]), f32(inputs["bso"])
    b1, b2 = f32(inputs["b1"]), f32(inputs["b2"])

    Wk1, Wv1 = Wkv[:, :D], Wkv[:, D:]
    Wq2, Wk2, Wv2 = Wqkv[:, :D], Wqkv[:, D:2 * D], Wqkv[:, 2 * D:]

    # LN affine folding: (x_hat * s + b) @ W = x_hat @ (diag(s) W) + b @ W.
    # K-projection bias drops (softmax shift invariance); V-projection bias
    # passes through row-normalized softmax and folds into the next bias.
    shared = {
        "wq": _tile_w(ln1_s[:, None] * Wq),
        "wk1": _tile_w(ln2_s[:, None] * Wk1),
        "wv1": _mov_w(ln2_s[:, None] * Wv1),
        "wco": _tile_w(Wco),
        "wq2": _tile_w(ln3_s[:, None] * Wq2),
        "wk2": _tile_w(ln3_s[:, None] * Wk2),
        "wv2": _mov_w(ln3_s[:, None] * Wv2),
        "wso": _tile_w(Wso),
        "w1": _tile_w(ln4_s[:, None] * W1),
        "w2": _tile_w(W2),
        "cols": np.ascontiguousarray(np.concatenate([
            _headpad_bias(ln1_b @ Wq),
            _headpad_bias(ln3_b @ Wq2),
            _halves_mask(),
            _colmaj(bco + (ln2_b @ Wv1) @ Wco),
            _colmaj(bso + (ln3_b @ Wv2) @ Wso),
            _colmaj(b1 + ln4_b @ W1),
            _colmaj(b2),
            _colmaj((ln2_s[:, None] * Wk1).sum(0)),
            _colmaj((ln3_s[:, None] * Wk2).sum(0)),
            _colmaj((ln4_s[:, None] * W1).sum(0)),
            _colmaj((ln1_s[:, None] * Wq).sum(0)),
        ], axis=1)),
    }
    in_maps = []
    for c in range(N_CORES):
        b, s = c // 2, c % 2
        m = dict(shared)
        xc = _chunk_fm(x[b, s * T:(s + 1) * T, :])
        m["xT"] = xc
        m["xbfT"] = xc.astype(ml_dtypes.bfloat16)
        m["kvT"] = _chunk_fm(
            key_val[b, s * T:(s + 1) * T, :]).astype(ml_dtypes.bfloat16)
        in_maps.append(m)

    res = run_bass_kernel_spmd(nc, in_maps, list(range(N_CORES)))
    _PROGRAM_CACHE["last_result"] = res

    out = np.empty((B, NSEQ, D), np.float32)
    for c in range(N_CORES):
        b, s = c // 2, c % 2
        o = np.asarray(res.results[c]["outT"], np.float32)  # [128, C8, T]
        out[b, s * T:(s + 1) * T, :] = o.transpose(2, 1, 0).reshape(T, D)
    return out
